# revision 36
# baseline (speedup 1.0000x reference)
"""DFFN Trainium2 kernel: proj_in 1x1 -> 8x8-patch rfft2*filt*irfft2 ->
gated GELU -> 1x1 -> depthwise 3x3 -> 1x1 -> +residual.

Data-parallel over batch: 8 images, one per NeuronCore.  ~200us/core
(TimelineSim) vs the 887us original; all engines within ~20% of each
other (PE/DVE/Act ~140-146us busy, DMA/Pool ~120us).

Key ideas:
  - bf16 I/O: x is cast to bf16 on the host (and pre-arranged per band
    in patch-major order, so proj_in's stationary operands are contiguous
    128-column slices) and the output DMAs back as bf16.  Halves HBM
    traffic; the branch is small and out = x + branch tolerates ~4e-3.
  - proj_in runs flipped (x 2-patch chunk stationary, w_in^T moving), so
    its output lands with patch pixels on partitions - the layout the
    per-channel FFT-filter maps M_c (blockdiag(Mc^T, Mc^T) matmuls)
    contract over.
  - The entire tail (w_before -> depthwise 3x3 -> w_out) is fused into
    C_k = W_o diag(w_dw[:,k]) W_b per tap, applied as fp8e4m3 DoubleRow
    matmuls (two taps per instruction at 0.5 cycles/row) whose moving
    operands are shifted reads of a halo'd fp8 g-slab; they accumulate
    straight into the 128 output channels in PSUM.  A DG_SCALE=64
    pre-scale keeps C_k out of the fp8 subnormal range and is divided
    back out by the PSUM eviction.
  - The +x residual is an SBUF-only tensor_add placed mostly on the
    otherwise-idle GPSIMD engine (which may not touch PSUM).
  - The whole thing is software-pipelined: step s emits A(s), B(s-1),
    T(s-2), tail(s-4) so every engine works on a different band; all
    PSUM tiles rotate through one shared 8-bank pool.

Walrus constraints found the hard way: matmul stationary APs allow only
one free dimension (moving APs are flexible), GPSIMD cannot access PSUM,
and InstTensorScalarPtr APs are limited to partition + 2 free dims.
"""

import sys

sys.path.insert(0, "/opt/trn_rl_repo")

import numpy as np
import ml_dtypes
from contextlib import ExitStack

import concourse.bass as bass
import concourse.mybir as mybir
import concourse.tile as tile
from concourse.bass_utils import run_bass_kernel_spmd
from concourse.masks import make_identity

F32 = mybir.dt.float32
BF16 = mybir.dt.bfloat16
FP8 = mybir.dt.float8e4
BF = ml_dtypes.bfloat16
E4M3 = ml_dtypes.float8_e4m3fn
DG_SCALE = 64.0

B, C, H, W = 8, 128, 256, 256
HALF = C // 2
P = 8
BAND = 16            # image rows per band
N_CORES = 8


# --------------------------------------------------------------------------
# host-side weight preprocessing
# --------------------------------------------------------------------------

def _prep_weights(fft_filt, w_in, w_before, w_dw, w_out):
    # M_c: per-channel 64x64 map patch -> irfft2(rfft2(patch) * filt_c).
    E = np.eye(P * P, dtype=np.float64).reshape(P * P, P, P)
    FB = np.fft.rfft2(E)                                    # [64, 8, 5]
    prod = FB[None] * fft_filt.astype(np.float64)[:, None]  # [C, 64, 8, 5]
    cols = np.fft.irfft2(prod, s=(P, P)).reshape(C, P * P, P * P)
    # cols[c, k, :] is column k of M_c, i.e. cols[c] = M_c^T = the lhsT we
    # need (lhsT[k_in, m_out] = M_c[m_out, k_in]).
    McT = cols  # [C, 64in, 64out]
    M2 = np.zeros((C, 128, 128), dtype=np.float64)
    M2[:, :64, :64] = McT
    M2[:, 64:, 64:] = McT
    # lhsT layout in SBUF: [128 part, C*128]
    m2_sb = np.ascontiguousarray(M2.transpose(1, 0, 2).reshape(128, C * 128))

    winT = np.ascontiguousarray(w_in.T)                     # [c_in, c_out]

    # c10[:, k*128:(k+1)*128] = C_k^T = W_b^T diag(w_dw[:,k]) W_o^T
    # (whole tail w_before -> dw tap k -> w_out as one 64->128 matrix),
    # duplicated on both partition halves so either slab half-slice can be
    # the matmul contraction.  fp8e4m3 scaled by DG_SCALE (values ~1e-3
    # would be subnormal unscaled); the psO eviction divides it back out.
    # Slot 8 = ZERO block (DoubleRow pairs with tap 8 in slot 9).
    wdw9 = w_dw.reshape(HALF, 9).astype(np.float64)
    wbT = w_before.T.astype(np.float64)                     # [cc_in, c_out]
    woT = w_out.T.astype(np.float64)                        # [cc, 128]
    c10 = np.zeros((128, 10 * 128), dtype=np.float64)
    for k in range(9):
        s9 = k if k < 8 else 9
        blk = (wbT * wdw9[None, :, k]) @ woT                # [64, 128]
        c10[:64, s9 * 128:(s9 + 1) * 128] = blk
        c10[64:, s9 * 128:(s9 + 1) * 128] = blk

    return (
        m2_sb.astype(BF),
        winT.astype(BF),
        (c10 * DG_SCALE).astype(E4M3),
    )


# --------------------------------------------------------------------------
# the tile kernel (per core, one image)
# --------------------------------------------------------------------------

def build_kernel(nc, n_rows=H, legalize=True,
                 act=mybir.ActivationFunctionType.Gelu, dev_rowwise=False):
    x_d = nc.dram_tensor("x", [C, n_rows, W], BF16, kind="ExternalInput").ap()
    m2_d = nc.dram_tensor("m2", [128, C * 128], BF16, kind="ExternalInput").ap()
    winT_d = nc.dram_tensor("winT", [C, C], BF16, kind="ExternalInput").ap()
    c10_d = nc.dram_tensor("c10", [128, 10 * 128], FP8, kind="ExternalInput").ap()
    out_d = nc.dram_tensor("out", [C, n_rows, W], BF16, kind="ExternalOutput").ap()

    n_bands = n_rows // BAND

    with tile.TileContext(nc) as tc, ExitStack() as ctx:
        singles = ctx.enter_context(tc.tile_pool(name="singles", bufs=1))
        xin_p = ctx.enter_context(tc.tile_pool(name="xin", bufs=7))
        abuf_p = ctx.enter_context(tc.tile_pool(name="abuf", bufs=3))
        gelu_p = ctx.enter_context(tc.tile_pool(name="gelu", bufs=2))
        g2_p = ctx.enter_context(tc.tile_pool(name="g2", bufs=3))
        slab_p = ctx.enter_context(tc.tile_pool(name="slab", bufs=4))
        outb_p = ctx.enter_context(tc.tile_pool(name="outb", bufs=2))

        ps_p = ctx.enter_context(tc.tile_pool(name="ps", bufs=8, space="PSUM"))

        # ---- load weights into SBUF once (m2 is 4MB; x-band DMAs are
        # issued first in the schedule so A(0) isn't blocked behind it) ----
        winT_sb = singles.tile([128, 128], BF16)
        nc.sync.dma_start(out=winT_sb, in_=winT_d)
        m2_sb = singles.tile([128, C * 128], BF16)
        c10_sb = singles.tile([128, 10 * 128], FP8)
        ident = singles.tile([128, 128], BF16)
        make_identity(nc, ident)

        slabs = []      # ring of per-band g slabs (with halo)
        xbands = []     # per-band bf16 x tiles (for residual)

        abufs = []
        gelus = []
        g2s = []

        def do_dma(t):
            y0 = t * BAND
            xband = xin_p.tile([128, BAND * W], BF16)
            nc.sync.dma_start(out=xband, in_=x_d[:, y0:y0 + BAND, :])
            xbands.append(xband)

        def do_A(t):
            """Stage A: proj_in, flipped (2-patch pixels on out parts).
            lhsT for pair (h2, w2) reads xband directly: cols (pl, i, j)."""
            xband = xbands[t]
            abuf = abuf_p.tile([128, C * 32], BF16)   # [comps, (c, pp)]
            abufs.append(abuf)
            for qg in range(8):
                psA = ps_p.tile([128, 512], F32, tag='ps')
                for q in range(4):
                    pp = qg * 4 + q
                    nc.tensor.matmul(
                        psA[:, q * 128:(q + 1) * 128],
                        xband[:, pp * 128:(pp + 1) * 128], winT_sb,
                        start=True, stop=True,
                    )
                # evict 4 chunks: psA cols (q, o) -> abuf cols o*32 + pp0+q
                pp0 = qg * 4
                dst = bass.AP(
                    tensor=abuf.tensor,
                    offset=abuf.offset + pp0,
                    ap=[abuf.ap[0], [1, 4], [32, 128]],
                )
                src = psA.rearrange("p (q o) -> p q o", q=4)
                if qg in (1, 3, 5):
                    nc.vector.tensor_copy(dst, src)
                else:
                    nc.scalar.copy(dst, src)

        def do_B(t):
            """Stage B: per-channel FFT-filter matmuls + gated GELU."""
            abuf = abufs[t]
            gelu_sb = gelu_p.tile([128, 4 * 512], BF16)
            g2 = g2_p.tile([128, 16 * 128], BF16)     # col = q*128 + xh*64 + cc
            gelus.append(gelu_sb)
            g2s.append(g2)
            for g in range(4):
                psB = ps_p.tile([128, 512], F32, tag='ps')
                for j in range(16):
                    c = g * 16 + j
                    nc.tensor.matmul(
                        psB[:, j * 32:(j + 1) * 32],
                        m2_sb[:, c * 128:(c + 1) * 128],
                        abuf[:, c * 32:(c + 1) * 32],
                        start=True, stop=True,
                    )
                nc.scalar.activation(
                    gelu_sb[:, g * 512:(g + 1) * 512], psB, act,
                )
                psB2 = ps_p.tile([128, 512], F32, tag='ps')
                for j in range(16):
                    c = 64 + g * 16 + j
                    nc.tensor.matmul(
                        psB2[:, j * 32:(j + 1) * 32],
                        m2_sb[:, c * 128:(c + 1) * 128],
                        abuf[:, c * 32:(c + 1) * 32],
                        start=True, stop=True,
                    )
                # gate into g2: col = (h2*8+w2')*128 + xh*64 + (g*16+j)
                dst = bass.AP(
                    tensor=g2.tensor,
                    offset=g2.offset + g * 16,
                    ap=[g2.ap[0], [64, 2], [1, 16], [1024, 2], [128, 8]],
                )
                src0 = bass.AP(
                    tensor=gelu_sb.tensor,
                    offset=gelu_sb.offset + g * 512,
                    ap=[gelu_sb.ap[0], [8, 2], [32, 16], [16, 2], [1, 8]],
                )
                src1 = bass.AP(
                    tensor=psB2.tensor,
                    offset=psB2.offset,
                    ap=[psB2.ap[0], [8, 2], [32, 16], [16, 2], [1, 8]],
                )
                nc.vector.tensor_mul(dst, src0, src1)

        def do_T(t):
            """Transpose to (xhalf, cc) partitions, scatter into the fp8
            halo slab (130-pitch rows, 1-px halo) in one pass."""
            g2 = g2s[t]
            slab = slab_p.tile([128, 18 * 130], FP8)
            slabs.append(slab)
            for h2 in range(2):
                psT = ps_p.tile([128, 1024], BF16, tag='ps')
                for w2p in range(8):
                    q = h2 * 8 + w2p
                    nc.tensor.transpose(
                        psT[:, w2p * 128:(w2p + 1) * 128],
                        g2[:, q * 128:(q + 1) * 128], ident)
                # psT col = w2p*128 + pl*64 + i*8 + j
                # -> slab col (1+8*h2+i)*130 + 1 + w2p*16 + pl*8 + j
                dst = bass.AP(
                    tensor=slab.tensor,
                    offset=slab.offset + (1 + 8 * h2) * 130 + 1,
                    ap=[slab.ap[0], [16, 8], [8, 2], [130, 8], [1, 8]],
                )
                src = psT.rearrange("p (w pl i j) -> p w pl i j", w=8, pl=2, i=8)
                if h2 == 0:
                    nc.vector.tensor_copy(dst, src)
                else:
                    nc.scalar.copy(dst, src)

            # zero the outer pad columns of rows 1..16 (image x=-1 / x=256)
            sl3 = slab.rearrange("p (r c) -> p r c", c=130)
            nc.gpsimd.memset(sl3[0:64, 1:17, 0:1], 0.0)
            nc.gpsimd.memset(sl3[64:128, 1:17, 129:130], 0.0)
            # seam: halo col 129 of left half <- col 1 of right half; col 0 of
            # right half <- col 128 of left half (rows 1..16)
            nc.sync.dma_start(out=sl3[0:64, 1:17, 129:130],
                              in_=sl3[64:128, 1:17, 1:2])
            nc.sync.dma_start(out=sl3[64:128, 1:17, 0:1],
                              in_=sl3[0:64, 1:17, 128:129])

            # halo rows between neighbouring bands
            if t == 0:
                nc.vector.memset(sl3[:, 0:1, :], 0.0)
            else:
                prev3 = slabs[t - 1].rearrange("p (r c) -> p r c", c=130)
                nc.gpsimd.tensor_copy(prev3[:, 17:18, :], sl3[:, 1:2, :])
                nc.gpsimd.tensor_copy(sl3[:, 0:1, :], prev3[:, 16:17, :])
            if t == n_bands - 1:
                nc.vector.memset(sl3[:, 17:18, :], 0.0)

        def do_DW(t, rowwise=False):
            """Fused tail: psO = sum_k C_k g(.+delta_k) * S  +  S*x, then
            evict with a 1/S scale into bf16 outb.  fp8 DoubleRow pairs
            contract the slab half (64 g-channels) straight into the 128
            output channels; the residual rides an S-scaled identity
            matmul whose moving operand reads patch-major x."""
            slab = slabs[t]
            y0 = t * BAND
            xband = xbands[t]
            outb = outb_p.tile([128, BAND * W], BF16)
            inv = 1.0 / DG_SCALE
            for ci in range(4):
                r0 = ci * 4
                h2 = r0 // 8
                for xh in range(2):
                    psO = ps_p.tile([128, 512], F32, tag='ps')
                    pslab = slab[xh * 64:(xh + 1) * 64, 0:1]
                    pc10 = c10_sb[xh * 64:(xh + 1) * 64, 0:1]
                    for p in range(5):          # DoubleRow tap pairs
                        if p < 4:
                            ka, kb = 2 * p, 2 * p + 1
                            da = (1 + r0 + ka // 3 - 1) * 130 + 1 + ka % 3 - 1
                            db = (1 + r0 + kb // 3 - 1) * 130 + 1 + kb % 3 - 1
                        else:
                            ka = 8              # zero block pairs with tap 8
                            db = (2 + r0) * 130 + 2
                            da = db - 130
                        lhsT = bass.AP(
                            tensor=c10_sb.tensor,
                            offset=pc10.offset + ka * 128,
                            ap=[pc10.ap[0], [128, 2], [1, 128]],
                        )
                        if rowwise:
                            for r in range(4):
                                rhs = bass.AP(
                                    tensor=slab.tensor,
                                    offset=pslab.offset + da + r * 130,
                                    ap=[pslab.ap[0], [db - da, 2], [1, 128]],
                                )
                                nc.tensor.matmul(
                                    psO[:, r * 128:(r + 1) * 128], lhsT, rhs,
                                    start=(p == 0), stop=False,
                                    perf_mode=mybir.MatmulPerfMode.DoubleRow,
                                    skip_group_check=True,
                                )
                        else:
                            rhs = bass.AP(
                                tensor=slab.tensor,
                                offset=pslab.offset + da,
                                ap=[pslab.ap[0], [db - da, 2], [130, 4],
                                    [1, 128]],
                            )
                            nc.tensor.matmul(
                                psO, lhsT, rhs,
                                start=(p == 0), stop=(p == 4),
                                perf_mode=mybir.MatmulPerfMode.DoubleRow,
                                skip_group_check=True,
                            )
                    osl = bass.AP(
                        tensor=outb.tensor,
                        offset=outb.offset + r0 * W + xh * 128,
                        ap=[outb.ap[0], [W, 4], [1, 128]],
                    )
                    src = psO.rearrange("p (r x) -> p r x", r=4)
                    if xh == 0:
                        nc.vector.tensor_scalar_mul(osl, src, inv)
                    else:
                        nc.scalar.mul(osl, src, inv)
                    # residual: outb += x, all-SBUF so Pool can carry it
                    # (GPSIMD may not touch PSUM); x is patch-major.
                    osl4 = bass.AP(
                        tensor=outb.tensor,
                        offset=outb.offset + r0 * W + xh * 128,
                        ap=[outb.ap[0], [W, 4], [16, 8], [8, 2], [1, 8]],
                    )
                    xsl = bass.AP(
                        tensor=xband.tensor,
                        offset=xband.offset + (h2 * 16 + 8 * xh) * 128
                        + (r0 % 8) * 8,
                        ap=[xband.ap[0], [8, 4], [128, 8], [64, 2], [1, 8]],
                    )
                    if (ci, xh) in ((0, 0), (2, 0)):
                        nc.vector.tensor_add(osl4, osl4, xsl)
                    else:
                        nc.gpsimd.tensor_add(osl4, osl4, xsl)
            nc.sync.dma_start(out=out_d[:, y0:y0 + BAND, :], in_=outb)

        # software-pipelined schedule: step s runs A(s) | B(s-1) | T(s-2) |
        # tail(s-4), with x DMA prefetched 2 steps ahead.  The gap between
        # T (slab scatter + seam DMAs + halo-row copies) and the tail that
        # reads the slab hides the ~3us seam-DMA latency.
        for s in range(n_bands + 5):
            if s == 0:
                do_dma(0)
                do_dma(1)
                nc.sync.dma_start(out=m2_sb, in_=m2_d)
                nc.sync.dma_start(out=c10_sb, in_=c10_d)
            if s + 2 < n_bands:
                do_dma(s + 2)
            if s < n_bands:
                do_A(s)
            if 0 <= s - 1 < n_bands:
                do_B(s - 1)
            if 0 <= s - 2 < n_bands:
                do_T(s - 2)
            if 0 <= s - 4 < n_bands:
                do_DW(s - 4, rowwise=dev_rowwise)

    if legalize:
        _spill_matmul_waits(nc)
    return nc


def _spill_matmul_waits(nc):
    """Walrus encodes at most ONE sync-wait per compute-engine ISA
    instruction.  Tile sometimes leaves 2+ waits on one instruction; split
    the extras into standalone EventSemaphore wait instructions inserted
    just before, on the same (in-order) engine queue."""
    import concourse.mybir as mb
    skip = (mb.InstEventSemaphore,)
    n = [0]
    for f in nc.m.functions:
        for bb in f.blocks:
            out = []
            for inst in bb.instructions:
                si = inst.sync_info
                if (si is not None and len(si.on_wait) > 1
                        and not isinstance(inst, skip)
                        and getattr(inst, 'engine', None) is not None):
                    extra, keep = si.on_wait[:-1], si.on_wait[-1:]
                    for w in extra:
                        n[0] += 1
                        carrier = mb.InstEventSemaphore(
                            name=f"I-waitfix-{n[0]}", ins=[], outs=[])
                        carrier.engine = inst.engine
                        carrier.sync_info = mb.SyncInfo(
                            on_wait=[w], on_update=[])
                        out.append(carrier)
                    si.on_wait = keep
                out.append(inst)
            bb.instructions = out


# --------------------------------------------------------------------------
# public entry point
# --------------------------------------------------------------------------

_CACHE = {}


def _get_nc():
    if "nc" not in _CACHE:
        nc = bass.Bass("TRN2", target_bir_lowering=False, debug=False)
        build_kernel(nc, n_rows=H)
        _CACHE["nc"] = nc
    return _CACHE["nc"]


def _reorder_x(img, n_rows=H):
    """[C, n_rows, W] row-major -> per-band patch-major:
    col (within band t) = (h2*16 + w2)*128 + pl*64 + i*8 + j."""
    c = img.reshape(C, n_rows // BAND, 2, 8, 16, 2, 8)  # c,t,h2,i,w2,pl,j
    return np.ascontiguousarray(
        c.transpose(0, 1, 2, 4, 5, 3, 6).reshape(C, n_rows, W))


def kernel(x, fft_filt, w_in, w_before, w_dw, w_out):
    x = np.asarray(x, dtype=np.float32).astype(BF)
    m2, winT, c10 = _prep_weights(
        np.asarray(fft_filt, np.float32), np.asarray(w_in, np.float32),
        np.asarray(w_before, np.float32), np.asarray(w_dw, np.float32),
        np.asarray(w_out, np.float32))

    nc = _get_nc()
    in_maps = []
    for i in range(N_CORES):
        in_maps.append({
            "x": _reorder_x(x[i]),
            "m2": m2, "winT": winT, "c10": c10,
        })
    res = run_bass_kernel_spmd(nc, in_maps, list(range(N_CORES)))
    out = np.stack([res.results[i]["out"] for i in range(N_CORES)], axis=0)
    return out.astype(np.float32)


# revision 37
# speedup vs baseline: 1.0407x; 1.0407x over previous
"""DFFN Trainium2 kernel: proj_in 1x1 -> 8x8-patch rfft2*filt*irfft2 ->
gated GELU -> 1x1 -> depthwise 3x3 -> 1x1 -> +residual.

Data-parallel over batch: 8 images, one per NeuronCore.  ~200us/core
(TimelineSim) vs the 887us original; all engines within ~20% of each
other (PE/DVE/Act ~140-146us busy, DMA/Pool ~120us).

Key ideas:
  - bf16 I/O: x is cast to bf16 on the host (and pre-arranged per band
    in patch-major order, so proj_in's stationary operands are contiguous
    128-column slices) and the output DMAs back as bf16.  Halves HBM
    traffic; the branch is small and out = x + branch tolerates ~4e-3.
  - proj_in runs flipped (x 2-patch chunk stationary, w_in^T moving), so
    its output lands with patch pixels on partitions - the layout the
    per-channel FFT-filter maps M_c (blockdiag(Mc^T, Mc^T) matmuls)
    contract over.
  - The entire tail (w_before -> depthwise 3x3 -> w_out) is fused into
    C_k = W_o diag(w_dw[:,k]) W_b per tap, applied as fp8e4m3 DoubleRow
    matmuls (two taps per instruction at 0.5 cycles/row) whose moving
    operands are shifted reads of a halo'd fp8 g-slab; they accumulate
    straight into the 128 output channels in PSUM.  A DG_SCALE=64
    pre-scale keeps C_k out of the fp8 subnormal range and is divided
    back out by the PSUM eviction.
  - The +x residual is an SBUF-only tensor_add placed mostly on the
    otherwise-idle GPSIMD engine (which may not touch PSUM).
  - The whole thing is software-pipelined: step s emits A(s), B(s-1),
    T(s-2), tail(s-4) so every engine works on a different band; all
    PSUM tiles rotate through one shared 8-bank pool.

Walrus constraints found the hard way: matmul stationary APs allow only
one free dimension (moving APs are flexible), GPSIMD cannot access PSUM,
and InstTensorScalarPtr APs are limited to partition + 2 free dims.
"""

import sys

sys.path.insert(0, "/opt/trn_rl_repo")

import numpy as np
import ml_dtypes
from contextlib import ExitStack

import concourse.bass as bass
import concourse.mybir as mybir
import concourse.tile as tile
from concourse.bass_utils import run_bass_kernel_spmd
from concourse.masks import make_identity

F32 = mybir.dt.float32
BF16 = mybir.dt.bfloat16
FP8 = mybir.dt.float8e4
BF = ml_dtypes.bfloat16
E4M3 = ml_dtypes.float8_e4m3fn
DG_SCALE = 64.0

B, C, H, W = 8, 128, 256, 256
HALF = C // 2
P = 8
BAND = 16            # image rows per band
N_CORES = 8


# --------------------------------------------------------------------------
# host-side weight preprocessing
# --------------------------------------------------------------------------

def _prep_weights(fft_filt, w_in, w_before, w_dw, w_out):
    # M_c: per-channel 64x64 map patch -> irfft2(rfft2(patch) * filt_c).
    E = np.eye(P * P, dtype=np.float64).reshape(P * P, P, P)
    FB = np.fft.rfft2(E)                                    # [64, 8, 5]
    prod = FB[None] * fft_filt.astype(np.float64)[:, None]  # [C, 64, 8, 5]
    cols = np.fft.irfft2(prod, s=(P, P)).reshape(C, P * P, P * P)
    # cols[c, k, :] is column k of M_c, i.e. cols[c] = M_c^T = the lhsT we
    # need (lhsT[k_in, m_out] = M_c[m_out, k_in]).
    McT = cols  # [C, 64in, 64out]
    M2 = np.zeros((C, 128, 128), dtype=np.float64)
    M2[:, :64, :64] = McT
    M2[:, 64:, 64:] = McT
    # lhsT layout in SBUF: [128 part, C*128]
    m2_sb = np.ascontiguousarray(M2.transpose(1, 0, 2).reshape(128, C * 128))

    winT = np.ascontiguousarray(w_in.T)                     # [c_in, c_out]

    # c10[:, k*128:(k+1)*128] = C_k^T = W_b^T diag(w_dw[:,k]) W_o^T
    # (whole tail w_before -> dw tap k -> w_out as one 64->128 matrix),
    # duplicated on both partition halves so either slab half-slice can be
    # the matmul contraction.  fp8e4m3 scaled by DG_SCALE (values ~1e-3
    # would be subnormal unscaled); the psO eviction divides it back out.
    # Slot 8 = ZERO block (DoubleRow pairs with tap 8 in slot 9).
    wdw9 = w_dw.reshape(HALF, 9).astype(np.float64)
    wbT = w_before.T.astype(np.float64)                     # [cc_in, c_out]
    woT = w_out.T.astype(np.float64)                        # [cc, 128]
    c10 = np.zeros((128, 10 * 128), dtype=np.float64)
    for k in range(9):
        s9 = k if k < 8 else 9
        blk = (wbT * wdw9[None, :, k]) @ woT                # [64, 128]
        c10[:64, s9 * 128:(s9 + 1) * 128] = blk
        c10[64:, s9 * 128:(s9 + 1) * 128] = blk

    return (
        m2_sb.astype(BF),
        winT.astype(BF),
        (c10 * DG_SCALE).astype(E4M3),
    )


# --------------------------------------------------------------------------
# the tile kernel (per core, one image)
# --------------------------------------------------------------------------

def build_kernel(nc, n_rows=H, legalize=True,
                 act=mybir.ActivationFunctionType.Gelu, dev_rowwise=False):
    x_d = nc.dram_tensor("x", [C, n_rows, W], BF16, kind="ExternalInput").ap()
    m2_d = nc.dram_tensor("m2", [128, C * 128], BF16, kind="ExternalInput").ap()
    winT_d = nc.dram_tensor("winT", [C, C], BF16, kind="ExternalInput").ap()
    c10_d = nc.dram_tensor("c10", [128, 10 * 128], FP8, kind="ExternalInput").ap()
    out_d = nc.dram_tensor("out", [C, n_rows, W], BF16, kind="ExternalOutput").ap()

    n_bands = n_rows // BAND

    with tile.TileContext(nc) as tc, ExitStack() as ctx:
        singles = ctx.enter_context(tc.tile_pool(name="singles", bufs=1))
        xin_p = ctx.enter_context(tc.tile_pool(name="xin", bufs=7))
        abuf_p = ctx.enter_context(tc.tile_pool(name="abuf", bufs=3))
        gelu_p = ctx.enter_context(tc.tile_pool(name="gelu", bufs=2))
        g2_p = ctx.enter_context(tc.tile_pool(name="g2", bufs=3))
        slab_p = ctx.enter_context(tc.tile_pool(name="slab", bufs=4))
        outb_p = ctx.enter_context(tc.tile_pool(name="outb", bufs=2))

        ps_p = ctx.enter_context(tc.tile_pool(name="ps", bufs=6, space="PSUM"))
        psa_p = ctx.enter_context(tc.tile_pool(name="psa", bufs=2, space="PSUM"))

        # ---- load weights into SBUF once (m2 is 4MB; x-band DMAs are
        # issued first in the schedule so A(0) isn't blocked behind it) ----
        winT_sb = singles.tile([128, 128], BF16)
        nc.sync.dma_start(out=winT_sb, in_=winT_d)
        m2_sb = singles.tile([128, C * 128], BF16)
        c10_sb = singles.tile([128, 10 * 128], FP8)
        ident = singles.tile([128, 128], BF16)
        make_identity(nc, ident)

        slabs = []      # ring of per-band g slabs (with halo)
        xbands = []     # per-band bf16 x tiles (for residual)

        abufs = []
        gelus = []
        g2s = []

        def do_dma(t):
            y0 = t * BAND
            xband = xin_p.tile([128, BAND * W], BF16)
            nc.sync.dma_start(out=xband, in_=x_d[:, y0:y0 + BAND, :])
            xbands.append(xband)

        def do_A(t):
            """Stage A: proj_in, flipped (2-patch pixels on out parts).
            lhsT for pair (h2, w2) reads xband directly: cols (pl, i, j)."""
            xband = xbands[t]
            abuf = abuf_p.tile([128, C * 32], BF16)   # [comps, (c, pp)]
            abufs.append(abuf)
            for qg in range(8):
                psA = psa_p.tile([128, 512], F32, tag='psa')
                for q in range(4):
                    pp = qg * 4 + q
                    nc.tensor.matmul(
                        psA[:, q * 128:(q + 1) * 128],
                        xband[:, pp * 128:(pp + 1) * 128], winT_sb,
                        start=True, stop=True,
                    )
                # evict 4 chunks: psA cols (q, o) -> abuf cols o*32 + pp0+q
                pp0 = qg * 4
                dst = bass.AP(
                    tensor=abuf.tensor,
                    offset=abuf.offset + pp0,
                    ap=[abuf.ap[0], [1, 4], [32, 128]],
                )
                src = psA.rearrange("p (q o) -> p q o", q=4)
                if qg in (1, 3, 5):
                    nc.vector.tensor_copy(dst, src)
                else:
                    nc.scalar.copy(dst, src)

        def do_B(t):
            """Stage B: per-channel FFT-filter matmuls + gated GELU."""
            abuf = abufs[t]
            gelu_sb = gelu_p.tile([128, 4 * 512], BF16)
            g2 = g2_p.tile([128, 16 * 128], BF16)     # col = q*128 + xh*64 + cc
            gelus.append(gelu_sb)
            g2s.append(g2)
            for g in range(4):
                psB = ps_p.tile([128, 512], F32, tag='ps')
                for j in range(16):
                    c = g * 16 + j
                    nc.tensor.matmul(
                        psB[:, j * 32:(j + 1) * 32],
                        m2_sb[:, c * 128:(c + 1) * 128],
                        abuf[:, c * 32:(c + 1) * 32],
                        start=True, stop=True,
                    )
                nc.scalar.activation(
                    gelu_sb[:, g * 512:(g + 1) * 512], psB, act,
                )
                psB2 = ps_p.tile([128, 512], F32, tag='ps')
                for j in range(16):
                    c = 64 + g * 16 + j
                    nc.tensor.matmul(
                        psB2[:, j * 32:(j + 1) * 32],
                        m2_sb[:, c * 128:(c + 1) * 128],
                        abuf[:, c * 32:(c + 1) * 32],
                        start=True, stop=True,
                    )
                # gate into g2: col = (h2*8+w2')*128 + xh*64 + (g*16+j)
                dst = bass.AP(
                    tensor=g2.tensor,
                    offset=g2.offset + g * 16,
                    ap=[g2.ap[0], [64, 2], [1, 16], [1024, 2], [128, 8]],
                )
                src0 = bass.AP(
                    tensor=gelu_sb.tensor,
                    offset=gelu_sb.offset + g * 512,
                    ap=[gelu_sb.ap[0], [8, 2], [32, 16], [16, 2], [1, 8]],
                )
                src1 = bass.AP(
                    tensor=psB2.tensor,
                    offset=psB2.offset,
                    ap=[psB2.ap[0], [8, 2], [32, 16], [16, 2], [1, 8]],
                )
                nc.vector.tensor_mul(dst, src0, src1)

        def do_T(t):
            """Transpose to (xhalf, cc) partitions, scatter into the fp8
            halo slab (130-pitch rows, 1-px halo) in one pass."""
            g2 = g2s[t]
            slab = slab_p.tile([128, 18 * 130], FP8)
            slabs.append(slab)
            for h2 in range(2):
                psT = ps_p.tile([128, 1024], BF16, tag='ps')
                for w2p in range(8):
                    q = h2 * 8 + w2p
                    nc.tensor.transpose(
                        psT[:, w2p * 128:(w2p + 1) * 128],
                        g2[:, q * 128:(q + 1) * 128], ident)
                # psT col = w2p*128 + pl*64 + i*8 + j
                # -> slab col (1+8*h2+i)*130 + 1 + w2p*16 + pl*8 + j
                dst = bass.AP(
                    tensor=slab.tensor,
                    offset=slab.offset + (1 + 8 * h2) * 130 + 1,
                    ap=[slab.ap[0], [16, 8], [8, 2], [130, 8], [1, 8]],
                )
                src = psT.rearrange("p (w pl i j) -> p w pl i j", w=8, pl=2, i=8)
                if h2 == 0:
                    nc.vector.tensor_copy(dst, src)
                else:
                    nc.scalar.copy(dst, src)

            # zero the outer pad columns of rows 1..16 (image x=-1 / x=256)
            sl3 = slab.rearrange("p (r c) -> p r c", c=130)
            nc.gpsimd.memset(sl3[0:64, 1:17, 0:1], 0.0)
            nc.gpsimd.memset(sl3[64:128, 1:17, 129:130], 0.0)
            # seam: halo col 129 of left half <- col 1 of right half; col 0 of
            # right half <- col 128 of left half (rows 1..16)
            nc.sync.dma_start(out=sl3[0:64, 1:17, 129:130],
                              in_=sl3[64:128, 1:17, 1:2])
            nc.sync.dma_start(out=sl3[64:128, 1:17, 0:1],
                              in_=sl3[0:64, 1:17, 128:129])

            # halo rows between neighbouring bands
            if t == 0:
                nc.vector.memset(sl3[:, 0:1, :], 0.0)
            else:
                prev3 = slabs[t - 1].rearrange("p (r c) -> p r c", c=130)
                nc.gpsimd.tensor_copy(prev3[:, 17:18, :], sl3[:, 1:2, :])
                nc.gpsimd.tensor_copy(sl3[:, 0:1, :], prev3[:, 16:17, :])
            if t == n_bands - 1:
                nc.vector.memset(sl3[:, 17:18, :], 0.0)

        def do_DW(t, rowwise=False):
            """Fused tail: psO = sum_k C_k g(.+delta_k) * S  +  S*x, then
            evict with a 1/S scale into bf16 outb.  fp8 DoubleRow pairs
            contract the slab half (64 g-channels) straight into the 128
            output channels; the residual rides an S-scaled identity
            matmul whose moving operand reads patch-major x."""
            slab = slabs[t]
            y0 = t * BAND
            xband = xbands[t]
            outb = outb_p.tile([128, BAND * W], BF16)
            inv = 1.0 / DG_SCALE
            for ci in range(4):
                r0 = ci * 4
                h2 = r0 // 8
                for xh in range(2):
                    psO = ps_p.tile([128, 512], F32, tag='ps')
                    pslab = slab[xh * 64:(xh + 1) * 64, 0:1]
                    pc10 = c10_sb[xh * 64:(xh + 1) * 64, 0:1]
                    for p in range(5):          # DoubleRow tap pairs
                        if p < 4:
                            ka, kb = 2 * p, 2 * p + 1
                            da = (1 + r0 + ka // 3 - 1) * 130 + 1 + ka % 3 - 1
                            db = (1 + r0 + kb // 3 - 1) * 130 + 1 + kb % 3 - 1
                        else:
                            ka = 8              # zero block pairs with tap 8
                            db = (2 + r0) * 130 + 2
                            da = db - 130
                        lhsT = bass.AP(
                            tensor=c10_sb.tensor,
                            offset=pc10.offset + ka * 128,
                            ap=[pc10.ap[0], [128, 2], [1, 128]],
                        )
                        if rowwise:
                            for r in range(4):
                                rhs = bass.AP(
                                    tensor=slab.tensor,
                                    offset=pslab.offset + da + r * 130,
                                    ap=[pslab.ap[0], [db - da, 2], [1, 128]],
                                )
                                nc.tensor.matmul(
                                    psO[:, r * 128:(r + 1) * 128], lhsT, rhs,
                                    start=(p == 0), stop=False,
                                    perf_mode=mybir.MatmulPerfMode.DoubleRow,
                                    skip_group_check=True,
                                )
                        else:
                            rhs = bass.AP(
                                tensor=slab.tensor,
                                offset=pslab.offset + da,
                                ap=[pslab.ap[0], [db - da, 2], [130, 4],
                                    [1, 128]],
                            )
                            nc.tensor.matmul(
                                psO, lhsT, rhs,
                                start=(p == 0), stop=(p == 4),
                                perf_mode=mybir.MatmulPerfMode.DoubleRow,
                                skip_group_check=True,
                            )
                    osl = bass.AP(
                        tensor=outb.tensor,
                        offset=outb.offset + r0 * W + xh * 128,
                        ap=[outb.ap[0], [W, 4], [1, 128]],
                    )
                    src = psO.rearrange("p (r x) -> p r x", r=4)
                    if xh == 0:
                        nc.vector.tensor_scalar_mul(osl, src, inv)
                    else:
                        nc.scalar.mul(osl, src, inv)
                    # residual: outb += x, all-SBUF so Pool can carry it
                    # (GPSIMD may not touch PSUM); x is patch-major.
                    osl4 = bass.AP(
                        tensor=outb.tensor,
                        offset=outb.offset + r0 * W + xh * 128,
                        ap=[outb.ap[0], [W, 4], [16, 8], [8, 2], [1, 8]],
                    )
                    xsl = bass.AP(
                        tensor=xband.tensor,
                        offset=xband.offset + (h2 * 16 + 8 * xh) * 128
                        + (r0 % 8) * 8,
                        ap=[xband.ap[0], [8, 4], [128, 8], [64, 2], [1, 8]],
                    )
                    if (ci, xh) in ((0, 0), (2, 0)):
                        nc.vector.tensor_add(osl4, osl4, xsl)
                    else:
                        nc.gpsimd.tensor_add(osl4, osl4, xsl)
            nc.sync.dma_start(out=out_d[:, y0:y0 + BAND, :], in_=outb)

        # software-pipelined schedule: step s runs A(s) | B(s-1) | T(s-2) |
        # tail(s-4), with x DMA prefetched 2 steps ahead.  The gap between
        # T (slab scatter + seam DMAs + halo-row copies) and the tail that
        # reads the slab hides the ~3us seam-DMA latency.
        for s in range(n_bands + 5):
            if s == 0:
                do_dma(0)
                do_dma(1)
                nc.sync.dma_start(out=m2_sb, in_=m2_d)
                nc.sync.dma_start(out=c10_sb, in_=c10_d)
            if s + 2 < n_bands:
                do_dma(s + 2)
            if s < n_bands:
                do_A(s)
            if 0 <= s - 1 < n_bands:
                do_B(s - 1)
            if 0 <= s - 2 < n_bands:
                do_T(s - 2)
            if 0 <= s - 4 < n_bands:
                do_DW(s - 4, rowwise=dev_rowwise)

    if legalize:
        _spill_matmul_waits(nc)
    return nc


def _spill_matmul_waits(nc):
    """Walrus encodes at most ONE sync-wait per compute-engine ISA
    instruction.  Tile sometimes leaves 2+ waits on one instruction; split
    the extras into standalone EventSemaphore wait instructions inserted
    just before, on the same (in-order) engine queue."""
    import concourse.mybir as mb
    skip = (mb.InstEventSemaphore,)
    n = [0]
    for f in nc.m.functions:
        for bb in f.blocks:
            out = []
            for inst in bb.instructions:
                si = inst.sync_info
                if (si is not None and len(si.on_wait) > 1
                        and not isinstance(inst, skip)
                        and getattr(inst, 'engine', None) is not None):
                    extra, keep = si.on_wait[:-1], si.on_wait[-1:]
                    for w in extra:
                        n[0] += 1
                        carrier = mb.InstEventSemaphore(
                            name=f"I-waitfix-{n[0]}", ins=[], outs=[])
                        carrier.engine = inst.engine
                        carrier.sync_info = mb.SyncInfo(
                            on_wait=[w], on_update=[])
                        out.append(carrier)
                    si.on_wait = keep
                out.append(inst)
            bb.instructions = out


# --------------------------------------------------------------------------
# public entry point
# --------------------------------------------------------------------------

_CACHE = {}


def _get_nc():
    if "nc" not in _CACHE:
        nc = bass.Bass("TRN2", target_bir_lowering=False, debug=False)
        build_kernel(nc, n_rows=H)
        _CACHE["nc"] = nc
    return _CACHE["nc"]


def _reorder_x(img, n_rows=H):
    """[C, n_rows, W] row-major -> per-band patch-major:
    col (within band t) = (h2*16 + w2)*128 + pl*64 + i*8 + j."""
    c = img.reshape(C, n_rows // BAND, 2, 8, 16, 2, 8)  # c,t,h2,i,w2,pl,j
    return np.ascontiguousarray(
        c.transpose(0, 1, 2, 4, 5, 3, 6).reshape(C, n_rows, W))


def kernel(x, fft_filt, w_in, w_before, w_dw, w_out):
    x = np.asarray(x, dtype=np.float32).astype(BF)
    m2, winT, c10 = _prep_weights(
        np.asarray(fft_filt, np.float32), np.asarray(w_in, np.float32),
        np.asarray(w_before, np.float32), np.asarray(w_dw, np.float32),
        np.asarray(w_out, np.float32))

    nc = _get_nc()
    in_maps = []
    for i in range(N_CORES):
        in_maps.append({
            "x": _reorder_x(x[i]),
            "m2": m2, "winT": winT, "c10": c10,
        })
    res = run_bass_kernel_spmd(nc, in_maps, list(range(N_CORES)))
    out = np.stack([res.results[i]["out"] for i in range(N_CORES)], axis=0)
    return out.astype(np.float32)
